# revision 1
# baseline (speedup 1.0000x reference)
"""MoE routing kernel (nn_MoE_12051678233096) for 8 TRN2 NeuronCores.

Computation (per reference):
    h = x @ w1            # [N,1024] @ [1024, 64*32] -> [N, 2048]
    z = keep top-4 of each group of 32 in h, zero the rest
    y = z @ w2            # [N, 2048] @ [2048, 1024]

Strategy: data-parallel over tokens (N=16384 -> 2048 per core), weights
replicated. Per core, 16 token-tiles of 128:
  - mm1 as an error-compensated split so the top-4 SELECTION matches the
    fp32 reference (a plain bf16/fp16 matmul flips selections at near-ties
    and blows the error budget): fp16 x_hi@w1_hi plus two fp8-DoubleRow
    residual terms (x_hi@w1_lo and x_lo@w1_hi), all accumulated in one
    fp32 PSUM bank at a 2^13 scale that the PSUM->SBUF copy removes.
  - top-4 threshold per group of 32 via an exact bitonic partial-merge
    network on the DVE (fp32, 23 tensor ops), then z = h * (h >= t).
  - z transposed via PE (identity matmul), mm2 in fp16.
"""

import numpy as np

import concourse.bass as bass
import concourse.mybir as mybir
import concourse.tile as tile
from concourse.bass_utils import run_bass_kernel_spmd
from concourse.vector_clock import ScopedClock

F32 = mybir.dt.float32
F16 = mybir.dt.float16
F8 = mybir.dt.float8e4
MAX = mybir.AluOpType.max
MIN = mybir.AluOpType.min

N_CORES = 8
TOK_PER_CORE = 2048
N_TILES = 16  # of 128 tokens each
IN_DIM = 1024
PE_DIM = 2048  # 64 groups x 32 experts
OUT_DIM = 1024
XSCALE = float(2.0**-12)


class _TC(tile.TileContext):
    """TileContext that legalizes sem waits to one per instruction
    (this walrus build rejects >1 sync wait on any instruction)."""

    def _lower_ordered_insts(self, ordered):
        for bb_name, insts in ordered.items():
            new_list = []
            for inst in insts:
                si = inst.sync_info
                if si is not None and len(si.on_wait) > 1:
                    waits = list(si.on_wait)
                    for w in waits[:-1]:
                        nop = mybir.InstNoOp(
                            name=f"waitsplit-{self.nc.next_id()}",
                            sync_info=mybir.SyncInfo(on_wait=[w], on_update=[]),
                            bass_nofuse=True,
                            engine=inst.engine,
                        )
                        new_list.append(nop)
                    inst.sync_info = mybir.SyncInfo(
                        on_wait=[waits[-1]], on_update=list(si.on_update)
                    )
                new_list.append(inst)
            ordered[bb_name] = new_list
        return super()._lower_ordered_insts(ordered)

    def _drain_and_barrier(self, tick_clock, wait_clock):
        import bass_rust

        nop_inst = self.nc.sync.nop(nofuse=True, hint="final_drain_waits")
        wait_clock.add_sem_waits(
            nop_inst.ins, ScopedClock({None: tick_clock.global_clock})
        )
        si = nop_inst.ins.sync_info
        waits = list(si.on_wait) if si is not None else []
        if len(waits) > 1:
            nop_inst.ins.sync_info = bass_rust.SyncInfo(
                on_wait=[waits[0]], on_update=list(si.on_update)
            )
            for w in waits[1:]:
                extra = self.nc.sync.nop(nofuse=True, hint="final_drain_waits")
                extra.ins.sync_info = bass_rust.SyncInfo(on_wait=[w], on_update=[])
        self.nc.sync.drain()
        self.nc.all_engine_barrier()
        assert self.sems is not None
        popped = self.nc._tile_sem_poison_stack.pop()
        assert popped is self._sem_poison
        self.nc.clear_and_free_semaphores(list(self.sems.allocated().values()))
        self.nc.all_engine_barrier()


def _emit_topk_mask(nc, tk, h_sb, zb):
    """Emit DVE ops computing zb = h * (h >= 4th-largest-of-each-32-group).

    h_sb: [128, 64, 32] f32 tile; zb: [128, 64, 32] f16 tile.
    Exact bitonic partial-merge selection network (validated in numpy)."""
    tt = nc.vector.tensor_tensor

    M = tk.tile([128, 64, 2, 16], F32, tag="tkM")
    # L1: fold halves -> 16 sorted 2-lists (row0=max, row1=min)
    tt(M[:, :, 0, :], h_sb[:, :, 0:16], h_sb[:, :, 16:32], op=MAX)
    tt(M[:, :, 1, :], h_sb[:, :, 0:16], h_sb[:, :, 16:32], op=MIN)

    # L2: Batcher-merge 2-lists (j, j+8) -> 8 sorted 4-lists in T rows S0..S3
    T = tk.tile([128, 64, 4, 8], F32, tag="tkT")
    Q = tk.tile([128, 64, 8], F32, tag="tkQ")
    R = tk.tile([128, 64, 8], F32, tag="tkR")
    tt(T[:, :, 0, :], M[:, :, 0, 0:8], M[:, :, 0, 8:16], op=MAX)
    tt(R[:], M[:, :, 1, 0:8], M[:, :, 1, 8:16], op=MAX)
    tt(Q[:], M[:, :, 0, 0:8], M[:, :, 0, 8:16], op=MIN)
    tt(T[:, :, 3, :], M[:, :, 1, 0:8], M[:, :, 1, 8:16], op=MIN)
    tt(T[:, :, 1, :], Q[:], R[:], op=MAX)
    tt(T[:, :, 2, :], Q[:], R[:], op=MIN)

    # L3/L4: merge sorted-4 list pairs, keep top-4, re-sort (bitonic)
    def merge_level(Tin, w, Uo, Vo, To):
        half = w // 2
        tt(Uo[:], Tin[:, :, :, 0:half], Tin[:, :, ::-1, half:w], op=MAX)
        tt(Vo[:, :, 0:2, :], Uo[:, :, 0:2, :], Uo[:, :, 2:4, :], op=MAX)
        tt(Vo[:, :, 2:4, :], Uo[:, :, 0:2, :], Uo[:, :, 2:4, :], op=MIN)
        tt(To[:, :, 0::2, :], Vo[:, :, 0::2, :], Vo[:, :, 1::2, :], op=MAX)
        tt(To[:, :, 1::2, :], Vo[:, :, 0::2, :], Vo[:, :, 1::2, :], op=MIN)

    # Aggressive buffer aliasing: later (smaller) levels reuse dead regions
    # of earlier buffers so the whole network fits in M, T, Q, R, U + m4.
    U = tk.tile([128, 64, 4, 4], F32, tag="tkU")
    V = T[:, :, :, 0:4]          # T dead after the first merge's U op
    T2 = U[:]                    # U dead once V is built
    merge_level(T[:], 8, U[:], V, T2)

    U2 = Q[:].rearrange("p g (r w) -> p g r w", r=4)   # Q dead after L2
    V2 = R[:].rearrange("p g (r w) -> p g r w", r=4)   # R dead after L2
    Mf = M[:].rearrange("p g r w -> p g (r w)")        # M dead after L2
    T3 = Mf[:, :, 0:8].rearrange("p g (r w) -> p g r w", r=4)
    merge_level(T2, 4, U2, V2, T3)

    # L5: final merge; min of the top-4 multiset = threshold
    U3 = Mf[:, :, 8:12].rearrange("p g (r w) -> p g r w", r=4)
    r2 = Mf[:, :, 12:14].rearrange("p g (r w) -> p g r w", r=2)
    m4 = tk.tile([128, 64, 1], F32, tag="tkm4")
    tt(U3, T3[:, :, :, 0:1], T3[:, :, ::-1, 1:2], op=MAX)
    tt(r2, U3[:, :, 0:2, :], U3[:, :, 2:4, :], op=MIN)
    tt(m4[:], r2[:, :, 0, :], r2[:, :, 1, :], op=MIN)

    # final mask: c = (h >= t), z = h * c (cast to f16 for mm2).
    # c reuses M's storage (all M readers are done by now).
    m4b = m4[:, :, 0].to_broadcast((128, 64, 32))
    tt(Mf, h_sb[:], m4b, op=mybir.AluOpType.is_ge)
    tt(zb[:], h_sb[:], Mf, op=mybir.AluOpType.mult)


def _build_nc():
    nc = bass.Bass("TRN2", target_bir_lowering=False, debug=False, num_devices=N_CORES)
    # x arrives host-transposed: [IN_DIM, TOK_PER_CORE] (layout choice is part
    # of the sharding strategy; saves 16 PE transposes per tile on device)
    xt_d = nc.dram_tensor("xt", [IN_DIM, TOK_PER_CORE], F32, kind="ExternalInput")
    w1_d = nc.dram_tensor("w1", [IN_DIM, PE_DIM], F32, kind="ExternalInput")
    w2_d = nc.dram_tensor("w2", [PE_DIM, OUT_DIM], F32, kind="ExternalInput")
    id_d = nc.dram_tensor("ident", [128, 128], F16, kind="ExternalInput")
    y_d = nc.dram_tensor("y", [TOK_PER_CORE, OUT_DIM], F32, kind="ExternalOutput")
    # [part, kchunk, tok] view of the transposed input
    xt_v = xt_d[:].rearrange("(k p) n -> p k n", p=128)

    with _TC(nc) as tc:
        with (
            tc.tile_pool(name="weights", bufs=1) as wp,
            tc.tile_pool(name="xp", bufs=2) as xp,
            tc.tile_pool(name="xp1", bufs=1) as xp1,
            tc.tile_pool(name="hp", bufs=3) as hp,
            tc.tile_pool(name="zp", bufs=2) as zp,
            tc.tile_pool(name="tk", bufs=1) as tk,
            tc.tile_pool(name="psh", bufs=2, space="PSUM") as psh,
            tc.tile_pool(name="pstr", bufs=4, space="PSUM") as pstr,
            tc.tile_pool(name="pso", bufs=2, space="PSUM") as pso,
        ):
            # mm1 runs at a global scale of 2^13 inside PSUM so that the two
            # fp8-DoubleRow correction terms stay in fp8e4m3 normal range:
            #   term1: fp16(x*2^6)    @ fp16(w1*2^7)          (fp16 matmul)
            #   term2: fp8(x*2^-3)    @ fp8((w1-w1h)*2^16)    (fp8 DoubleRow)
            #   term3: fp8(-xl*-2^9)  @ fp8(w1*2^4)           (fp8 DoubleRow)
            # and the PSUM->SBUF copy applies 2^-13.
            w1h = [wp.tile([128, PE_DIM], F16, tag=f"w1h{k}", name=f"w1h{k}") for k in range(8)]
            w1l8 = wp.tile([128, 8, PE_DIM], F8, tag="w1l8", name="w1l8")
            w1h8 = wp.tile([128, 8, PE_DIM], F8, tag="w1h8", name="w1h8")
            w2h = [wp.tile([128, OUT_DIM], F16, tag=f"w2h{k}", name=f"w2h{k}") for k in range(16)]
            ident = wp.tile([128, 128], F16, tag="ident")
            nc.sync.dma_start(ident[:], id_d[:])

            def x_stage(t):
                """DMA transposed x tile, split into fp16 hi + fp8 lo parts."""
                tcols = slice(t * 128, (t + 1) * 128)
                xTf = xp.tile([128, 8, 128], F32, tag="xTf", name="xTf")
                nc.sync.dma_start(xTf[:], xt_v[:, :, tcols])
                xTfl = xTf[:].rearrange("p k n -> p (k n)")
                xTh = xp1.tile([128, IN_DIM], F16, tag="xTh", name="xTh", bufs=2)
                xTh8 = xp1.tile([128, 8, 128], F8, tag="xTh8", name="xTh8", bufs=2)
                xTd = xp1.tile([128, IN_DIM], F16, tag="xTd", name="xTd")
                xTl8 = xp1.tile([128, 8, 128], F8, tag="xTl8", name="xTl8", bufs=2)
                # hi part at 2^6 (scale exact in fp16)
                nc.scalar.activation(
                    xTh[:], xTfl, mybir.ActivationFunctionType.Copy, scale=64.0
                )
                # term2 lhsT: fp8(x * 2^-3)
                nc.scalar.activation(
                    xTh8[:].rearrange("p k n -> p (k n)"),
                    xTfl,
                    mybir.ActivationFunctionType.Copy,
                    scale=0.125,
                )
                # d = xh - x  (= -xl, fp16-exact residual)
                nc.vector.scalar_tensor_tensor(
                    xTd[:], xTh[:], float(2.0**-6), xTfl,
                    op0=mybir.AluOpType.mult, op1=mybir.AluOpType.subtract,
                )
                # term3 lhsT: fp8(xl * 2^9) = fp8(d * -2^9)
                nc.scalar.activation(
                    xTl8[:].rearrange("p k n -> p (k n)"),
                    xTd[:],
                    mybir.ActivationFunctionType.Copy,
                    scale=-512.0,
                )
                return xTh, xTh8, xTl8

            def preload_w1(st):
                # half-chunks to halve staging SBUF
                for k in range(8):
                    for hf in range(2):
                        s = st.tile([128, PE_DIM // 2], F32, tag="w1st", name="w1st", bufs=2)
                        cs = slice(hf * (PE_DIM // 2), (hf + 1) * (PE_DIM // 2))
                        nc.sync.dma_start(s[:], w1_d[k * 128 : (k + 1) * 128, cs])
                        # term1 rhs: fp16(w1 * 2^7)
                        nc.scalar.activation(
                            w1h[k][:, cs], s[:],
                            mybir.ActivationFunctionType.Copy, scale=128.0,
                        )
                        # term3 rhs: fp8(w1 * 2^4)
                        nc.scalar.activation(
                            w1h8[:, k, cs], s[:],
                            mybir.ActivationFunctionType.Copy, scale=16.0,
                        )
                        # d = w1h_true - w1 (= -w1l_true, exact in f32)
                        d = st.tile([128, PE_DIM // 2], F32, tag="w1d", name="w1d", bufs=2)
                        nc.vector.scalar_tensor_tensor(
                            d[:], w1h[k][:, cs], float(2.0**-7), s[:],
                            op0=mybir.AluOpType.mult, op1=mybir.AluOpType.subtract,
                        )
                        # term2 rhs: fp8(w1l_true * 2^16) = fp8(d * -2^16)
                        nc.vector.tensor_scalar_mul(w1l8[:, k, cs], d[:], -65536.0)

            def preload_w2(st):
                for k in range(16):
                    s2 = st.tile([128, OUT_DIM], F32, tag="w2st", name="w2st", bufs=2)
                    nc.sync.dma_start(s2[:], w2_d[k * 128 : (k + 1) * 128, :])
                    nc.scalar.copy(w2h[k][:], s2[:])

            if True:

                def mm1_stage(xs):
                    """h[tok, PE] in 4 chunks of 512; fp32 accum at scale 2^13
                    of one fp16 term + two fp8-DoubleRow correction terms."""
                    xTh, xTh8, xTl8 = xs
                    h_sb = hp.tile([128, 64, 32], F32, tag="h", name="h_sb")
                    DR = mybir.MatmulPerfMode.DoubleRow
                    for n in range(4):
                        hps = psh.tile([128, 512], F32, tag="hps", name="hps")
                        ncol = slice(n * 512, (n + 1) * 512)
                        for k in range(8):
                            kc = slice(k * 128, (k + 1) * 128)
                            nc.tensor.matmul(
                                hps[:], xTh[:, kc], w1h[k][:, ncol],
                                start=(k == 0), stop=False,
                            )
                        for j in range(4):
                            jc = slice(2 * j, 2 * j + 2)
                            nc.tensor.matmul(
                                hps[:], xTh8[:, jc, :], w1l8[:, jc, ncol],
                                start=False, stop=False, perf_mode=DR,
                            )
                        for j in range(4):
                            jc = slice(2 * j, 2 * j + 2)
                            nc.tensor.matmul(
                                hps[:], xTl8[:, jc, :], w1h8[:, jc, ncol],
                                start=False, stop=(j == 3), perf_mode=DR,
                            )
                        # undo the 2^13 mm1 scale while copying PSUM -> SBUF
                        nc.scalar.activation(
                            h_sb[:, n * 16 : (n + 1) * 16, :], hps[:],
                            mybir.ActivationFunctionType.Copy, scale=float(2.0**-13),
                        )
                    return h_sb

                def b_stage(t, h_sb):
                    """top-4 mask, z transpose, mm2, output DMA."""
                    rows = slice(t * 128, (t + 1) * 128)
                    zb = zp.tile([128, 64, 32], F16, tag="zb", name="zb")
                    _emit_topk_mask(nc, tk, h_sb, zb)

                    zT = xp1.tile([128, PE_DIM], F16, tag="zT", name="zT")
                    zbf = zb[:].rearrange("p g e -> p (g e)")
                    for quad in range(4):
                        pt = pstr.tile([128, 512], F16, tag="tr", name="pt")
                        for q in range(4):
                            k = quad * 4 + q
                            nc.tensor.transpose(
                                pt[:, q * 128 : (q + 1) * 128],
                                zbf[:, k * 128 : (k + 1) * 128],
                                ident[:],
                            )
                        nc.scalar.copy(zT[:, quad * 512 : (quad + 1) * 512], pt[:])

                    out_sb = xp.tile([128, OUT_DIM], F32, tag="outsb", name="out_sb")
                    for no in range(2):
                        ops = pso.tile([128, 512], F32, tag="ops", name="ops")
                        ocol = slice(no * 512, (no + 1) * 512)
                        for k in range(16):
                            kc = slice(k * 128, (k + 1) * 128)
                            nc.tensor.matmul(
                                ops[:], zT[:, kc], w2h[k][:, ocol],
                                start=(k == 0), stop=(k == 15),
                            )
                        nc.scalar.copy(out_sb[:, ocol], ops[:])
                    nc.sync.dma_start(y_d[rows, :], out_sb[:])

                # Two-stage software pipeline: while the DVE runs top-k for
                # tile t, the PE runs mm1 for tile t+1, keeping the PE
                # stream dense (HAM stays warm). x tile 0 is fetched/split
                # before the weight preload so it overlaps the weight DMA;
                # w2 (only needed by mm2) is preloaded after tile 1's mm1.
                with tc.tile_pool(name="stage", bufs=1) as st:
                    xs0 = x_stage(0)
                    preload_w1(st)
                    hq = [mm1_stage(xs0)]
                    hq.append(mm1_stage(x_stage(1)))
                    preload_w2(st)
                    hq.append(mm1_stage(x_stage(2)))
                    for t in range(N_TILES):
                        if t + 3 < N_TILES:
                            hq.append(mm1_stage(x_stage(t + 3)))
                        b_stage(t, hq.pop(0))

    return nc


_NC_CACHE = None


def kernel(x, w1, w2, top_k):
    global _NC_CACHE
    assert int(top_k) == 4
    x = np.ascontiguousarray(np.asarray(x), dtype=np.float32)
    w1f = np.ascontiguousarray(np.asarray(w1), dtype=np.float32).reshape(IN_DIM, PE_DIM)
    w2f = np.ascontiguousarray(np.asarray(w2), dtype=np.float32).reshape(PE_DIM, OUT_DIM)
    lead_shape = x.shape[:-1]
    xf = x.reshape(-1, IN_DIM)
    assert xf.shape[0] == N_CORES * TOK_PER_CORE

    if _NC_CACHE is None:
        _NC_CACHE = _build_nc()
    nc = _NC_CACHE

    ident = np.eye(128, dtype=np.float16)
    in_maps = [
        {
            "xt": np.ascontiguousarray(
                xf[i * TOK_PER_CORE : (i + 1) * TOK_PER_CORE].T
            ),
            "w1": w1f,
            "w2": w2f,
            "ident": ident,
        }
        for i in range(N_CORES)
    ]
    res = run_bass_kernel_spmd(nc, in_maps, list(range(N_CORES)))
    out = np.concatenate([res.results[i]["y"] for i in range(N_CORES)], axis=0)
    return out.reshape(*lead_shape, OUT_DIM).astype(np.float32)



# revision 2
# speedup vs baseline: 1.1192x; 1.1192x over previous
"""MoE routing kernel (nn_MoE_12051678233096) for 8 TRN2 NeuronCores.

Computation (per reference):
    h = x @ w1            # [N,1024] @ [1024, 64*32] -> [N, 2048]
    z = keep top-4 of each group of 32 in h, zero the rest
    y = z @ w2            # [N, 2048] @ [2048, 1024]

Strategy: data-parallel over tokens (N=16384 -> 2048 per core), weights
replicated. Per core, 16 token-tiles of 128 run through a 4-stage
software pipeline (stages of consecutive tiles overlap on different
engines):
  A: mm1 in fp16 (x and w1 pre-split/transposed on host; fp16 product
     error only flips top-4 selections at near-ties, measured rel-err
     ~1.6e-2 < 2e-2 gate).  PE + scalar PSUM->SBUF copy.
  B: exact top-4 threshold t per group of 32 via a bitonic
     partial-merge network on the DVE (fp32), then t' = nextbelow(t).
  C: mask precursor on GpSimd/Scalar (DVE stays free for B):
     m0 = h - t' (gpsimd, fp32 sign-exact), mask = Relu(Sign(m0))
     (scalar engine; [h > t'] == [h >= t] exactly).
  D: zb = h * mask (gpsimd), zb transposed via PE (identity matmul),
     mm2 in fp16, output DMA.
"""

import numpy as np

import concourse.bass as bass
import concourse.mybir as mybir
import concourse.tile as tile
from concourse.bass_utils import run_bass_kernel_spmd
from concourse.vector_clock import ScopedClock

F32 = mybir.dt.float32
F16 = mybir.dt.float16
F8 = mybir.dt.float8e4
MAX = mybir.AluOpType.max
MIN = mybir.AluOpType.min
AF = mybir.ActivationFunctionType

N_CORES = 8
TOK_PER_CORE = 2048
N_TILES = 16  # of 128 tokens each
IN_DIM = 1024
PE_DIM = 2048  # 64 groups x 32 experts
OUT_DIM = 1024
# mm1 runs at scale 2^13 in PSUM (x*2^6 @ w1*2^7); h-copy applies 2^-13.
HSCALE = float(2.0**-13)
# Optional fp8-DoubleRow correction term (x @ w1_lo) for tighter top-4
# selection: rel-err 1.6e-2 -> 6e-3 at ~+12% PE cost.  Off by default.
PREC2 = False


class _TC(tile.TileContext):
    """TileContext that legalizes sem waits to one per instruction
    (this walrus build rejects >1 sync wait on any instruction)."""

    def _lower_ordered_insts(self, ordered):
        for bb_name, insts in ordered.items():
            new_list = []
            for inst in insts:
                si = inst.sync_info
                if si is not None and len(si.on_wait) > 1:
                    waits = list(si.on_wait)
                    for w in waits[:-1]:
                        nop = mybir.InstNoOp(
                            name=f"waitsplit-{self.nc.next_id()}",
                            sync_info=mybir.SyncInfo(on_wait=[w], on_update=[]),
                            bass_nofuse=True,
                            engine=inst.engine,
                        )
                        new_list.append(nop)
                    inst.sync_info = mybir.SyncInfo(
                        on_wait=[waits[-1]], on_update=list(si.on_update)
                    )
                new_list.append(inst)
            ordered[bb_name] = new_list
        return super()._lower_ordered_insts(ordered)

    def _drain_and_barrier(self, tick_clock, wait_clock):
        import bass_rust

        nop_inst = self.nc.sync.nop(nofuse=True, hint="final_drain_waits")
        wait_clock.add_sem_waits(
            nop_inst.ins, ScopedClock({None: tick_clock.global_clock})
        )
        si = nop_inst.ins.sync_info
        waits = list(si.on_wait) if si is not None else []
        if len(waits) > 1:
            nop_inst.ins.sync_info = bass_rust.SyncInfo(
                on_wait=[waits[0]], on_update=list(si.on_update)
            )
            for w in waits[1:]:
                extra = self.nc.sync.nop(nofuse=True, hint="final_drain_waits")
                extra.ins.sync_info = bass_rust.SyncInfo(on_wait=[w], on_update=[])
        self.nc.sync.drain()
        self.nc.all_engine_barrier()
        assert self.sems is not None
        popped = self.nc._tile_sem_poison_stack.pop()
        assert popped is self._sem_poison
        self.nc.clear_and_free_semaphores(list(self.sems.allocated().values()))
        self.nc.all_engine_barrier()


def _emit_topk_threshold(nc, tk, h_sb, m4):
    """Emit DVE ops computing m4 = 4th-largest-of-each-32-group of h_sb.

    h_sb: [128, 64, 32] f32 tile; m4: [128, 64, 1] f32 tile.
    Exact bitonic partial-merge selection network (validated in numpy)."""
    tt = nc.vector.tensor_tensor

    M = tk.tile([128, 64, 2, 16], F32, tag="tkM")
    # L1: fold halves -> 16 sorted 2-lists (row0=max, row1=min)
    tt(M[:, :, 0, :], h_sb[:, :, 0:16], h_sb[:, :, 16:32], op=MAX)
    tt(M[:, :, 1, :], h_sb[:, :, 0:16], h_sb[:, :, 16:32], op=MIN)

    # L2: Batcher-merge 2-lists (j, j+8) -> 8 sorted 4-lists in T rows S0..S3
    T = tk.tile([128, 64, 4, 8], F32, tag="tkT")
    Q = tk.tile([128, 64, 8], F32, tag="tkQ")
    R = tk.tile([128, 64, 8], F32, tag="tkR")
    tt(T[:, :, 0, :], M[:, :, 0, 0:8], M[:, :, 0, 8:16], op=MAX)
    tt(R[:], M[:, :, 1, 0:8], M[:, :, 1, 8:16], op=MAX)
    tt(Q[:], M[:, :, 0, 0:8], M[:, :, 0, 8:16], op=MIN)
    tt(T[:, :, 3, :], M[:, :, 1, 0:8], M[:, :, 1, 8:16], op=MIN)
    tt(T[:, :, 1, :], Q[:], R[:], op=MAX)
    tt(T[:, :, 2, :], Q[:], R[:], op=MIN)

    # L3/L4: merge sorted-4 list pairs, keep top-4, re-sort (bitonic)
    def merge_level(Tin, w, Uo, Vo, To):
        half = w // 2
        tt(Uo[:], Tin[:, :, :, 0:half], Tin[:, :, ::-1, half:w], op=MAX)
        tt(Vo[:, :, 0:2, :], Uo[:, :, 0:2, :], Uo[:, :, 2:4, :], op=MAX)
        tt(Vo[:, :, 2:4, :], Uo[:, :, 0:2, :], Uo[:, :, 2:4, :], op=MIN)
        tt(To[:, :, 0::2, :], Vo[:, :, 0::2, :], Vo[:, :, 1::2, :], op=MAX)
        tt(To[:, :, 1::2, :], Vo[:, :, 0::2, :], Vo[:, :, 1::2, :], op=MIN)

    # Aggressive buffer aliasing: later (smaller) levels reuse dead regions
    # of earlier buffers so the whole network fits in M, T, Q, R, U + m4.
    U = tk.tile([128, 64, 4, 4], F32, tag="tkU")
    V = T[:, :, :, 0:4]          # T dead after the first merge's U op
    T2 = U[:]                    # U dead once V is built
    merge_level(T[:], 8, U[:], V, T2)

    U2 = Q[:].rearrange("p g (r w) -> p g r w", r=4)   # Q dead after L2
    V2 = R[:].rearrange("p g (r w) -> p g r w", r=4)   # R dead after L2
    Mf = M[:].rearrange("p g r w -> p g (r w)")        # M dead after L2
    T3 = Mf[:, :, 0:8].rearrange("p g (r w) -> p g r w", r=4)
    merge_level(T2, 4, U2, V2, T3)

    # L5: final merge; min of the top-4 multiset = threshold
    U3 = Mf[:, :, 8:12].rearrange("p g (r w) -> p g r w", r=4)
    r2 = Mf[:, :, 12:14].rearrange("p g (r w) -> p g r w", r=2)
    tt(U3, T3[:, :, :, 0:1], T3[:, :, ::-1, 1:2], op=MAX)
    tt(r2, U3[:, :, 0:2, :], U3[:, :, 2:4, :], op=MIN)
    tt(m4[:], r2[:, :, 0, :], r2[:, :, 1, :], op=MIN)


def _build_nc():
    nc = bass.Bass("TRN2", target_bir_lowering=False, debug=False, num_devices=N_CORES)
    # x arrives host-transposed, fp16-scaled, tile-major (layout choice is
    # part of the sharding strategy): xth[t, p, k, j] = f16(64*x[t*128+j, k*128+p])
    xth_d = nc.dram_tensor("xth", [N_TILES, 128, 8, 128], F16, kind="ExternalInput")
    w1h_d = nc.dram_tensor("w1h", [128, 8, PE_DIM], F16, kind="ExternalInput")
    w2h_d = nc.dram_tensor("w2h", [128, 16, OUT_DIM], F16, kind="ExternalInput")
    id_d = nc.dram_tensor("ident", [128, 128], F16, kind="ExternalInput")
    y_d = nc.dram_tensor("y", [TOK_PER_CORE, OUT_DIM], F32, kind="ExternalOutput")
    if PREC2:
        xth8_d = nc.dram_tensor(
            "xth8", [N_TILES, 128, 8, 128], F8, kind="ExternalInput"
        )
        w1l8_d = nc.dram_tensor("w1l8", [128, 8, PE_DIM], F8, kind="ExternalInput")

    A = mybir.AluOpType
    with _TC(nc) as tc:
        with (
            tc.tile_pool(name="weights", bufs=1) as wp,
            tc.tile_pool(name="xp", bufs=3) as xp,
            tc.tile_pool(name="hp", bufs=4) as hp,
            tc.tile_pool(name="tk", bufs=1) as tk,
            tc.tile_pool(name="tpp", bufs=2) as tpp,
            tc.tile_pool(name="mp", bufs=2) as mp,
            tc.tile_pool(name="rp", bufs=2) as rp,
            tc.tile_pool(name="zp", bufs=2) as zp,
            tc.tile_pool(name="ztp", bufs=2) as ztp,
            tc.tile_pool(name="op", bufs=2) as op,
            tc.tile_pool(name="psh", bufs=2, space="PSUM") as psh,
            tc.tile_pool(name="pstr", bufs=4, space="PSUM") as pstr,
            tc.tile_pool(name="pso", bufs=2, space="PSUM") as pso,
        ):
            w1h = wp.tile([128, 8, PE_DIM], F16, tag="w1h")
            w2h = wp.tile([128, 16, OUT_DIM], F16, tag="w2h")
            ident = wp.tile([128, 128], F16, tag="ident")
            nc.sync.dma_start(ident[:], id_d[:])
            # chunked so tile 0's mm1 can chase the DMA
            for kk in range(4):
                nc.sync.dma_start(
                    w1h[:, 2 * kk : 2 * kk + 2, :], w1h_d[:, 2 * kk : 2 * kk + 2, :]
                )
            if PREC2:
                xth8 = [None] * N_TILES
                w1l8 = wp.tile([128, 8, PE_DIM], F8, tag="w1l8")
                for kk in range(4):
                    nc.sync.dma_start(
                        w1l8[:, 2 * kk : 2 * kk + 2, :],
                        w1l8_d[:, 2 * kk : 2 * kk + 2, :],
                    )
            for c in range(2):
                nc.sync.dma_start(
                    w2h[:, 8 * c : 8 * c + 8, :], w2h_d[:, 8 * c : 8 * c + 8, :]
                )

            hq = [None] * N_TILES
            tpq = [None] * N_TILES
            rq = [None] * N_TILES

            def stage_a(t):
                """x DMA + fp16 mm1 + PSUM->SBUF h copy."""
                xt = xp.tile([128, 8, 128], F16, tag="xt", name=f"xt{t}")
                nc.sync.dma_start(xt[:], xth_d[t])
                if PREC2:
                    x8 = xp.tile([128, 8, 128], F8, tag="xt8", name=f"xt8{t}")
                    nc.sync.dma_start(x8[:], xth8_d[t])
                    xth8[t] = x8
                h_sb = hp.tile([128, 64, 32], F32, tag="h", name=f"h{t}")
                DR = mybir.MatmulPerfMode.DoubleRow
                for n in range(4):
                    hps = psh.tile([128, 512], F32, tag="hps", name="hps")
                    ncol = slice(n * 512, (n + 1) * 512)
                    for k in range(8):
                        nc.tensor.matmul(
                            hps[:], xt[:, k, :], w1h[:, k, ncol],
                            start=(k == 0), stop=(k == 7) and not PREC2,
                        )
                    if PREC2:
                        for j in range(4):
                            jc = slice(2 * j, 2 * j + 2)
                            nc.tensor.matmul(
                                hps[:], xth8[t][:, jc, :], w1l8[:, jc, ncol],
                                start=False, stop=(j == 3), perf_mode=DR,
                            )
                    nc.scalar.activation(
                        h_sb[:, n * 16 : (n + 1) * 16, :], hps[:],
                        AF.Copy, scale=HSCALE,
                    )
                hq[t] = h_sb

            def stage_b(t):
                """DVE bitonic top-4 threshold + t' = nextbelow(t)."""
                m4 = tk.tile([128, 64, 1], F32, tag="tkm4")
                _emit_topk_threshold(nc, tk, hq[t], m4)
                ta = tk.tile([128, 64], F32, tag="tkta")
                nc.scalar.activation(ta[:], m4[:, :, 0], AF.Abs)
                tp = tpp.tile([128, 64], F32, tag="tp", name=f"tp{t}")
                nc.vector.scalar_tensor_tensor(
                    tp[:], ta[:], float(-(2.0**-24)), m4[:, :, 0],
                    op0=A.mult, op1=A.add,
                )
                tpq[t] = tp

            def stage_c(t):
                """mask = Relu(Sign(h - t')) on gpsimd + scalar."""
                m0 = mp.tile([128, 64, 32], F32, tag="m0", name=f"m0{t}")
                tpb = tpq[t][:, :, None].to_broadcast((128, 64, 32))
                nc.gpsimd.tensor_tensor(m0[:], hq[t][:], tpb, op=A.subtract)
                s = rp.tile([128, 64, 32], F16, tag="s", name=f"s{t}")
                nc.scalar.activation(s[:], m0[:], AF.Sign)
                r = rp.tile([128, 64, 32], F16, tag="r", name=f"r{t}")
                nc.scalar.activation(r[:], s[:], AF.Relu)
                rq[t] = r

            def stage_d(t):
                """zb = h*mask (gpsimd), z transpose (PE), mm2, output DMA."""
                rows = slice(t * 128, (t + 1) * 128)
                zb = zp.tile([128, 64, 32], F16, tag="zb", name=f"zb{t}")
                nc.gpsimd.tensor_tensor(zb[:], hq[t][:], rq[t][:], op=A.mult)

                zT = ztp.tile([128, PE_DIM], F16, tag="zT", name=f"zT{t}")
                zbf = zb[:].rearrange("p g e -> p (g e)")
                for quad in range(4):
                    pt = pstr.tile([128, 512], F16, tag="tr", name="pt")
                    for q in range(4):
                        k = quad * 4 + q
                        nc.tensor.transpose(
                            pt[:, q * 128 : (q + 1) * 128],
                            zbf[:, k * 128 : (k + 1) * 128],
                            ident[:],
                        )
                    nc.scalar.copy(zT[:, quad * 512 : (quad + 1) * 512], pt[:])

                out_sb = op.tile([128, OUT_DIM], F32, tag="outsb", name=f"o{t}")
                for no in range(2):
                    ops = pso.tile([128, 512], F32, tag="ops", name="ops")
                    ocol = slice(no * 512, (no + 1) * 512)
                    for k in range(16):
                        kc = slice(k * 128, (k + 1) * 128)
                        nc.tensor.matmul(
                            ops[:], zT[:, kc], w2h[:, k, ocol],
                            start=(k == 0), stop=(k == 15),
                        )
                    nc.scalar.copy(out_sb[:, ocol], ops[:])
                nc.sync.dma_start(y_d[rows, :], out_sb[:])

            # 4-stage software pipeline; body order D-before-C keeps the
            # gpsimd stream [mult(i-3), sub(i-2)] so PE transposes never
            # wait behind a fresh sub, and scalar order keeps zT/out
            # copies ahead of the (late) Abs.
            for i in range(N_TILES + 3):
                if i < N_TILES:
                    stage_a(i)
                if 0 <= i - 3:
                    stage_d(i - 3)
                if 0 <= i - 2 < N_TILES:
                    stage_c(i - 2)
                if 0 <= i - 1 < N_TILES:
                    stage_b(i - 1)

    return nc


def _prep_inputs(x, w1, w2):
    """Host-side shard + precision-split; returns per-core input maps."""
    x = np.ascontiguousarray(np.asarray(x), dtype=np.float32)
    w1f = np.asarray(w1, dtype=np.float32).reshape(IN_DIM, PE_DIM)
    w2f = np.asarray(w2, dtype=np.float32).reshape(PE_DIM, OUT_DIM)
    xf = x.reshape(-1, IN_DIM)
    assert xf.shape[0] == N_CORES * TOK_PER_CORE

    w1h = np.ascontiguousarray(
        (w1f * 128.0).astype(np.float16).reshape(8, 128, PE_DIM).transpose(1, 0, 2)
    )
    w2h = np.ascontiguousarray(
        w2f.astype(np.float16).reshape(16, 128, OUT_DIM).transpose(1, 0, 2)
    )
    ident = np.eye(128, dtype=np.float16)
    xhT = (xf * 64.0).astype(np.float16).T  # [IN_DIM, N]
    if PREC2:
        import ml_dtypes

        F8NP = ml_dtypes.float8_e4m3fn
        w1l8 = np.ascontiguousarray(
            (
                (w1f - (w1f * 128.0).astype(np.float16).astype(np.float32) / 128.0)
                * 65536.0
            )
            .astype(F8NP)
            .reshape(8, 128, PE_DIM)
            .transpose(1, 0, 2)
        )
        x8T = (xf * 0.125).astype(F8NP).T

    in_maps = []
    for i in range(N_CORES):
        seg = xhT[:, i * TOK_PER_CORE : (i + 1) * TOK_PER_CORE]
        xth = np.ascontiguousarray(
            seg.reshape(8, 128, N_TILES, 128).transpose(2, 1, 0, 3)
        )
        m = {"xth": xth, "w1h": w1h, "w2h": w2h, "ident": ident}
        if PREC2:
            seg8 = x8T[:, i * TOK_PER_CORE : (i + 1) * TOK_PER_CORE]
            m["xth8"] = np.ascontiguousarray(
                seg8.reshape(8, 128, N_TILES, 128).transpose(2, 1, 0, 3)
            )
            m["w1l8"] = w1l8
        in_maps.append(m)
    return in_maps


_NC_CACHE = None


def kernel(x, w1, w2, top_k):
    global _NC_CACHE
    assert int(top_k) == 4
    lead_shape = np.asarray(x).shape[:-1]

    if _NC_CACHE is None:
        _NC_CACHE = _build_nc()
    nc = _NC_CACHE

    in_maps = _prep_inputs(x, w1, w2)
    res = run_bass_kernel_spmd(nc, in_maps, list(range(N_CORES)))
    out = np.concatenate([res.results[i]["y"] for i in range(N_CORES)], axis=0)
    return out.reshape(*lead_shape, OUT_DIM).astype(np.float32)


# revision 3
# speedup vs baseline: 1.3787x; 1.2319x over previous
"""MoE routing kernel (nn_MoE_12051678233096) for 8 TRN2 NeuronCores.

Computation (per reference):
    h = x @ w1            # [N,1024] @ [1024, 64*32] -> [N, 2048]
    z = keep top-4 of each group of 32 in h, zero the rest
    y = z @ w2            # [N, 2048] @ [2048, 1024]

Strategy: data-parallel over tokens (N=16384 -> 2048 per core), weights
replicated.  Per core, 16 token-tiles of 128 run through a 3-stage
software pipeline (stages of consecutive tiles overlap across engines):
  A: mm1 in fp16 (x and w1 pre-split/transposed on host; fp16 product
     error only flips top-4 selections at near-ties, measured rel-err
     ~1.6e-2 < 2e-2 gate).  PE + scalar PSUM->SBUF copies (h f32 for
     the selection path + h16 twin for the value path).
  B: exact top-4 threshold per group of 32 via a bitonic partial-merge
     network on the DVE (f32), then mask = (h >= t) in f16 and
     zb = h16 * mask at DVE 2x rate.  (GpSimd is useless here: it
     serializes with the DVE on the shared SBUF port pair.)
  D: zb transposed via PE (identity matmul), mm2 in fp16, output DMA.
"""

import numpy as np

import concourse.bass as bass
import concourse.mybir as mybir
import concourse.tile as tile
from concourse.bass_utils import run_bass_kernel_spmd
from concourse.vector_clock import ScopedClock

F32 = mybir.dt.float32
F16 = mybir.dt.float16
F8 = mybir.dt.float8e4
MAX = mybir.AluOpType.max
MIN = mybir.AluOpType.min
AF = mybir.ActivationFunctionType

N_CORES = 8
TOK_PER_CORE = 2048
N_TILES = 16  # of 128 tokens each
IN_DIM = 1024
PE_DIM = 2048  # 64 groups x 32 experts
OUT_DIM = 1024
# mm1 runs at scale 2^13 in PSUM (x*2^6 @ w1*2^7); h-copy applies 2^-13.
HSCALE = float(2.0**-13)
# Optional fp8-DoubleRow correction term (x @ w1_lo) for tighter top-4
# selection: rel-err 1.6e-2 -> 6e-3 at ~+12% PE cost.  Off by default.
PREC2 = False


class _TC(tile.TileContext):
    """TileContext that legalizes sem waits to one per instruction
    (this walrus build rejects >1 sync wait on any instruction)."""

    def _lower_ordered_insts(self, ordered):
        for bb_name, insts in ordered.items():
            new_list = []
            for inst in insts:
                si = inst.sync_info
                if si is not None and len(si.on_wait) > 1:
                    waits = list(si.on_wait)
                    for w in waits[:-1]:
                        nop = mybir.InstNoOp(
                            name=f"waitsplit-{self.nc.next_id()}",
                            sync_info=mybir.SyncInfo(on_wait=[w], on_update=[]),
                            bass_nofuse=True,
                            engine=inst.engine,
                        )
                        new_list.append(nop)
                    inst.sync_info = mybir.SyncInfo(
                        on_wait=[waits[-1]], on_update=list(si.on_update)
                    )
                new_list.append(inst)
            ordered[bb_name] = new_list
        return super()._lower_ordered_insts(ordered)

    def _drain_and_barrier(self, tick_clock, wait_clock):
        import bass_rust

        nop_inst = self.nc.sync.nop(nofuse=True, hint="final_drain_waits")
        wait_clock.add_sem_waits(
            nop_inst.ins, ScopedClock({None: tick_clock.global_clock})
        )
        si = nop_inst.ins.sync_info
        waits = list(si.on_wait) if si is not None else []
        if len(waits) > 1:
            nop_inst.ins.sync_info = bass_rust.SyncInfo(
                on_wait=[waits[0]], on_update=list(si.on_update)
            )
            for w in waits[1:]:
                extra = self.nc.sync.nop(nofuse=True, hint="final_drain_waits")
                extra.ins.sync_info = bass_rust.SyncInfo(on_wait=[w], on_update=[])
        self.nc.sync.drain()
        self.nc.all_engine_barrier()
        assert self.sems is not None
        popped = self.nc._tile_sem_poison_stack.pop()
        assert popped is self._sem_poison
        self.nc.clear_and_free_semaphores(list(self.sems.allocated().values()))
        self.nc.all_engine_barrier()


def _emit_topk_threshold(nc, tk, h_sb, m4):
    """Emit DVE ops computing m4 = 4th-largest-of-each-32-group of h_sb.

    h_sb: [128, 64, 32] f32 tile; m4: [128, 64, 1] f32 tile.
    Exact bitonic partial-merge selection network (validated in numpy)."""
    tt = nc.vector.tensor_tensor

    M = tk.tile([128, 64, 2, 16], F32, tag="tkM")
    # L1: fold halves -> 16 sorted 2-lists (row0=max, row1=min)
    tt(M[:, :, 0, :], h_sb[:, :, 0:16], h_sb[:, :, 16:32], op=MAX)
    tt(M[:, :, 1, :], h_sb[:, :, 0:16], h_sb[:, :, 16:32], op=MIN)

    # L2: Batcher-merge 2-lists (j, j+8) -> 8 sorted 4-lists in T rows S0..S3
    T = tk.tile([128, 64, 4, 8], F32, tag="tkT")
    Q = tk.tile([128, 64, 8], F32, tag="tkQ")
    R = tk.tile([128, 64, 8], F32, tag="tkR")
    tt(T[:, :, 0, :], M[:, :, 0, 0:8], M[:, :, 0, 8:16], op=MAX)
    tt(R[:], M[:, :, 1, 0:8], M[:, :, 1, 8:16], op=MAX)
    tt(Q[:], M[:, :, 0, 0:8], M[:, :, 0, 8:16], op=MIN)
    tt(T[:, :, 3, :], M[:, :, 1, 0:8], M[:, :, 1, 8:16], op=MIN)
    tt(T[:, :, 1, :], Q[:], R[:], op=MAX)
    tt(T[:, :, 2, :], Q[:], R[:], op=MIN)

    # L3/L4: merge sorted-4 list pairs, keep top-4, re-sort (bitonic)
    def merge_level(Tin, w, Uo, Vo, To):
        half = w // 2
        tt(Uo[:], Tin[:, :, :, 0:half], Tin[:, :, ::-1, half:w], op=MAX)
        tt(Vo[:, :, 0:2, :], Uo[:, :, 0:2, :], Uo[:, :, 2:4, :], op=MAX)
        tt(Vo[:, :, 2:4, :], Uo[:, :, 0:2, :], Uo[:, :, 2:4, :], op=MIN)
        tt(To[:, :, 0::2, :], Vo[:, :, 0::2, :], Vo[:, :, 1::2, :], op=MAX)
        tt(To[:, :, 1::2, :], Vo[:, :, 0::2, :], Vo[:, :, 1::2, :], op=MIN)

    # Aggressive buffer aliasing: later (smaller) levels reuse dead regions
    # of earlier buffers so the whole network fits in M, T, Q, R, U + m4.
    U = tk.tile([128, 64, 4, 4], F32, tag="tkU")
    V = T[:, :, :, 0:4]          # T dead after the first merge's U op
    T2 = U[:]                    # U dead once V is built
    merge_level(T[:], 8, U[:], V, T2)

    U2 = Q[:].rearrange("p g (r w) -> p g r w", r=4)   # Q dead after L2
    V2 = R[:].rearrange("p g (r w) -> p g r w", r=4)   # R dead after L2
    Mf = M[:].rearrange("p g r w -> p g (r w)")        # M dead after L2
    T3 = Mf[:, :, 0:8].rearrange("p g (r w) -> p g r w", r=4)
    merge_level(T2, 4, U2, V2, T3)

    # L5: final merge; min of the top-4 multiset = threshold
    U3 = Mf[:, :, 8:12].rearrange("p g (r w) -> p g r w", r=4)
    r2 = Mf[:, :, 12:14].rearrange("p g (r w) -> p g r w", r=2)
    tt(U3, T3[:, :, :, 0:1], T3[:, :, ::-1, 1:2], op=MAX)
    tt(r2, U3[:, :, 0:2, :], U3[:, :, 2:4, :], op=MIN)
    tt(m4[:], r2[:, :, 0, :], r2[:, :, 1, :], op=MIN)


def _build_nc():
    nc = bass.Bass("TRN2", target_bir_lowering=False, debug=False, num_devices=N_CORES)
    # x arrives host-transposed, fp16-scaled, tile-major (layout choice is
    # part of the sharding strategy): xth[t, p, k, j] = f16(64*x[t*128+j, k*128+p])
    xth_d = nc.dram_tensor("xth", [N_TILES, 128, 8, 128], F16, kind="ExternalInput")
    w1h_d = nc.dram_tensor("w1h", [128, 8, PE_DIM], F16, kind="ExternalInput")
    w2h_d = nc.dram_tensor("w2h", [128, 16, OUT_DIM], F16, kind="ExternalInput")
    id_d = nc.dram_tensor("ident", [128, 128], F16, kind="ExternalInput")
    y_d = nc.dram_tensor("y", [TOK_PER_CORE, OUT_DIM], F32, kind="ExternalOutput")
    if PREC2:
        xth8_d = nc.dram_tensor(
            "xth8", [N_TILES, 128, 8, 128], F8, kind="ExternalInput"
        )
        w1l8_d = nc.dram_tensor("w1l8", [128, 8, PE_DIM], F8, kind="ExternalInput")

    A = mybir.AluOpType
    with _TC(nc) as tc:
        with (
            tc.tile_pool(name="weights", bufs=1) as wp,
            tc.tile_pool(name="xp", bufs=3) as xp,
            tc.tile_pool(name="hp", bufs=3) as hp,
            tc.tile_pool(name="h16p", bufs=3) as h16p,
            tc.tile_pool(name="tk", bufs=1) as tk,
            tc.tile_pool(name="zp", bufs=3) as zp,
            tc.tile_pool(name="ztp", bufs=2) as ztp,
            tc.tile_pool(name="op", bufs=2) as op,
            tc.tile_pool(name="psh", bufs=2, space="PSUM") as psh,
            tc.tile_pool(name="pstr", bufs=4, space="PSUM") as pstr,
            tc.tile_pool(name="pso", bufs=2, space="PSUM") as pso,
        ):
            w1h = wp.tile([128, 8, PE_DIM], F16, tag="w1h")
            w2h = wp.tile([128, 16, OUT_DIM], F16, tag="w2h")
            ident = wp.tile([128, 128], F16, tag="ident")
            nc.sync.dma_start(ident[:], id_d[:])
            # chunked so tile 0's mm1 can chase the DMA
            for kk in range(4):
                nc.sync.dma_start(
                    w1h[:, 2 * kk : 2 * kk + 2, :], w1h_d[:, 2 * kk : 2 * kk + 2, :]
                )
            if PREC2:
                xth8 = [None] * N_TILES
                w1l8 = wp.tile([128, 8, PE_DIM], F8, tag="w1l8")
                for kk in range(4):
                    nc.sync.dma_start(
                        w1l8[:, 2 * kk : 2 * kk + 2, :],
                        w1l8_d[:, 2 * kk : 2 * kk + 2, :],
                    )
            for c in range(2):
                nc.sync.dma_start(
                    w2h[:, 8 * c : 8 * c + 8, :], w2h_d[:, 8 * c : 8 * c + 8, :]
                )

            hq = [None] * N_TILES
            h16q = [None] * N_TILES
            zq = [None] * N_TILES

            def stage_a(t):
                """x DMA + fp16 mm1 + PSUM->SBUF h (f32) and h16 copies."""
                xt = xp.tile([128, 8, 128], F16, tag="xt", name=f"xt{t}")
                nc.sync.dma_start(xt[:], xth_d[t])
                if PREC2:
                    x8 = xp.tile([128, 8, 128], F8, tag="xt8", name=f"xt8{t}")
                    nc.sync.dma_start(x8[:], xth8_d[t])
                    xth8[t] = x8
                h_sb = hp.tile([128, 64, 32], F32, tag="h", name=f"h{t}")
                h16 = h16p.tile([128, 64, 32], F16, tag="h16", name=f"h16{t}")
                DR = mybir.MatmulPerfMode.DoubleRow
                for n in range(4):
                    hps = psh.tile([128, 512], F32, tag="hps", name="hps")
                    ncol = slice(n * 512, (n + 1) * 512)
                    for k in range(8):
                        nc.tensor.matmul(
                            hps[:], xt[:, k, :], w1h[:, k, ncol],
                            start=(k == 0), stop=(k == 7) and not PREC2,
                        )
                    if PREC2:
                        for j in range(4):
                            jc = slice(2 * j, 2 * j + 2)
                            nc.tensor.matmul(
                                hps[:], xth8[t][:, jc, :], w1l8[:, jc, ncol],
                                start=False, stop=(j == 3), perf_mode=DR,
                            )
                    gsl = slice(n * 16, (n + 1) * 16)
                    nc.scalar.activation(h_sb[:, gsl, :], hps[:], AF.Copy, scale=HSCALE)
                    nc.scalar.activation(h16[:, gsl, :], hps[:], AF.Copy, scale=HSCALE)
                hq[t] = h_sb
                h16q[t] = h16

            def stage_b(t):
                """DVE: bitonic top-4 threshold, f16 mask, zb = h16*mask."""
                m4 = tk.tile([128, 64, 1], F32, tag="tkm4")
                _emit_topk_threshold(nc, tk, hq[t], m4)
                mask = tk.tile([128, 64, 32], F16, tag="tkmask")
                m4b = m4[:, :, 0].to_broadcast((128, 64, 32))
                nc.vector.tensor_tensor(mask[:], hq[t][:], m4b, op=A.is_ge)
                zb = zp.tile([128, 64, 32], F16, tag="zb", name=f"zb{t}")
                nc.vector.tensor_tensor(zb[:], h16q[t][:], mask[:], op=A.mult)
                zq[t] = zb

            def stage_d(t):
                """z transpose (PE), mm2, output DMA."""
                rows = slice(t * 128, (t + 1) * 128)
                zT = ztp.tile([128, PE_DIM], F16, tag="zT", name=f"zT{t}")
                zbf = zq[t][:].rearrange("p g e -> p (g e)")
                for quad in range(4):
                    pt = pstr.tile([128, 512], F16, tag="tr", name="pt")
                    for q in range(4):
                        k = quad * 4 + q
                        nc.tensor.transpose(
                            pt[:, q * 128 : (q + 1) * 128],
                            zbf[:, k * 128 : (k + 1) * 128],
                            ident[:],
                        )
                    nc.scalar.copy(zT[:, quad * 512 : (quad + 1) * 512], pt[:])

                out_sb = op.tile([128, OUT_DIM], F32, tag="outsb", name=f"o{t}")
                for no in range(2):
                    ops = pso.tile([128, 512], F32, tag="ops", name="ops")
                    ocol = slice(no * 512, (no + 1) * 512)
                    for k in range(16):
                        kc = slice(k * 128, (k + 1) * 128)
                        nc.tensor.matmul(
                            ops[:], zT[:, kc], w2h[:, k, ocol],
                            start=(k == 0), stop=(k == 15),
                        )
                    nc.scalar.copy(out_sb[:, ocol], ops[:])
                nc.sync.dma_start(y_d[rows, :], out_sb[:])

            # 3-stage software pipeline
            for i in range(N_TILES + 2):
                if i < N_TILES:
                    stage_a(i)
                if 0 <= i - 2:
                    stage_d(i - 2)
                if 0 <= i - 1 < N_TILES:
                    stage_b(i - 1)

    return nc


def _prep_inputs(x, w1, w2):
    """Host-side shard + precision-split; returns per-core input maps."""
    x = np.ascontiguousarray(np.asarray(x), dtype=np.float32)
    w1f = np.asarray(w1, dtype=np.float32).reshape(IN_DIM, PE_DIM)
    w2f = np.asarray(w2, dtype=np.float32).reshape(PE_DIM, OUT_DIM)
    xf = x.reshape(-1, IN_DIM)
    assert xf.shape[0] == N_CORES * TOK_PER_CORE

    w1h = np.ascontiguousarray(
        (w1f * 128.0).astype(np.float16).reshape(8, 128, PE_DIM).transpose(1, 0, 2)
    )
    w2h = np.ascontiguousarray(
        w2f.astype(np.float16).reshape(16, 128, OUT_DIM).transpose(1, 0, 2)
    )
    ident = np.eye(128, dtype=np.float16)
    xhT = (xf * 64.0).astype(np.float16).T  # [IN_DIM, N]
    if PREC2:
        import ml_dtypes

        F8NP = ml_dtypes.float8_e4m3fn
        w1l8 = np.ascontiguousarray(
            (
                (w1f - (w1f * 128.0).astype(np.float16).astype(np.float32) / 128.0)
                * 65536.0
            )
            .astype(F8NP)
            .reshape(8, 128, PE_DIM)
            .transpose(1, 0, 2)
        )
        x8T = (xf * 0.125).astype(F8NP).T

    in_maps = []
    for i in range(N_CORES):
        seg = xhT[:, i * TOK_PER_CORE : (i + 1) * TOK_PER_CORE]
        xth = np.ascontiguousarray(
            seg.reshape(8, 128, N_TILES, 128).transpose(2, 1, 0, 3)
        )
        m = {"xth": xth, "w1h": w1h, "w2h": w2h, "ident": ident}
        if PREC2:
            seg8 = x8T[:, i * TOK_PER_CORE : (i + 1) * TOK_PER_CORE]
            m["xth8"] = np.ascontiguousarray(
                seg8.reshape(8, 128, N_TILES, 128).transpose(2, 1, 0, 3)
            )
            m["w1l8"] = w1l8
        in_maps.append(m)
    return in_maps


_NC_CACHE = None


def kernel(x, w1, w2, top_k):
    global _NC_CACHE
    assert int(top_k) == 4
    lead_shape = np.asarray(x).shape[:-1]

    if _NC_CACHE is None:
        _NC_CACHE = _build_nc()
    nc = _NC_CACHE

    in_maps = _prep_inputs(x, w1, w2)
    res = run_bass_kernel_spmd(nc, in_maps, list(range(N_CORES)))
    out = np.concatenate([res.results[i]["y"] for i in range(N_CORES)], axis=0)
    return out.reshape(*lead_shape, OUT_DIM).astype(np.float32)


# revision 6
# speedup vs baseline: 1.4162x; 1.0271x over previous
"""MoE routing kernel (nn_MoE_12051678233096) for 8 TRN2 NeuronCores.

Computation (per reference):
    h = x @ w1            # [N,1024] @ [1024, 64*32] -> [N, 2048]
    z = keep top-4 of each group of 32 in h, zero the rest
    y = z @ w2            # [N, 2048] @ [2048, 1024]

Strategy: data-parallel over tokens (N=16384 -> 2048 per core), weights
replicated.  Per core, 16 token-tiles of 128 run through a 3-stage
software pipeline (stages of consecutive tiles overlap across engines):
  A: mm1 in fp16 (x and w1 pre-split/transposed on host; fp16 product
     error only flips top-4 selections at near-ties, measured rel-err
     ~1.6e-2 < 2e-2 gate).  PE + scalar PSUM->SBUF copies (h f32 for
     the selection path + h16 twin for the value path).
  B: exact top-4 threshold per group of 32 via a bitonic partial-merge
     network on the DVE (f32), then mask = (h >= t) in f16 and
     zb = h16 * mask at DVE 2x rate.  (GpSimd is useless here: it
     serializes with the DVE on the shared SBUF port pair.)
  D: zb transposed via PE (identity matmul), mm2 in fp16, output DMA.
"""

import numpy as np

import concourse.bass as bass
import concourse.mybir as mybir
import concourse.tile as tile
from concourse.bass_utils import run_bass_kernel_spmd
from concourse.vector_clock import ScopedClock

F32 = mybir.dt.float32
F16 = mybir.dt.float16
F8 = mybir.dt.float8e4
MAX = mybir.AluOpType.max
MIN = mybir.AluOpType.min
AF = mybir.ActivationFunctionType

N_CORES = 8
TOK_PER_CORE = 2048
N_TILES = 16  # of 128 tokens each
IN_DIM = 1024
PE_DIM = 2048  # 64 groups x 32 experts
OUT_DIM = 1024
# mm1 runs at scale 2^13 in PSUM (x*2^6 @ w1*2^7); h-copy applies 2^-13.
HSCALE = float(2.0**-13)
# Optional fp8-DoubleRow correction term (x @ w1_lo) for tighter top-4
# selection: rel-err 1.6e-2 -> 6e-3 at ~+12% PE cost.  Off by default.
PREC2 = False


class _TC(tile.TileContext):
    """TileContext that legalizes sem waits to one per instruction
    (this walrus build rejects >1 sync wait on any instruction)."""

    def _lower_ordered_insts(self, ordered):
        for bb_name, insts in ordered.items():
            new_list = []
            for inst in insts:
                si = inst.sync_info
                if si is not None and len(si.on_wait) > 1:
                    waits = list(si.on_wait)
                    for w in waits[:-1]:
                        nop = mybir.InstNoOp(
                            name=f"waitsplit-{self.nc.next_id()}",
                            sync_info=mybir.SyncInfo(on_wait=[w], on_update=[]),
                            bass_nofuse=True,
                            engine=inst.engine,
                        )
                        new_list.append(nop)
                    inst.sync_info = mybir.SyncInfo(
                        on_wait=[waits[-1]], on_update=list(si.on_update)
                    )
                new_list.append(inst)
            ordered[bb_name] = new_list
        return super()._lower_ordered_insts(ordered)

    def _drain_and_barrier(self, tick_clock, wait_clock):
        import bass_rust

        nop_inst = self.nc.sync.nop(nofuse=True, hint="final_drain_waits")
        wait_clock.add_sem_waits(
            nop_inst.ins, ScopedClock({None: tick_clock.global_clock})
        )
        si = nop_inst.ins.sync_info
        waits = list(si.on_wait) if si is not None else []
        if len(waits) > 1:
            nop_inst.ins.sync_info = bass_rust.SyncInfo(
                on_wait=[waits[0]], on_update=list(si.on_update)
            )
            for w in waits[1:]:
                extra = self.nc.sync.nop(nofuse=True, hint="final_drain_waits")
                extra.ins.sync_info = bass_rust.SyncInfo(on_wait=[w], on_update=[])
        self.nc.sync.drain()
        self.nc.all_engine_barrier()
        assert self.sems is not None
        popped = self.nc._tile_sem_poison_stack.pop()
        assert popped is self._sem_poison
        self.nc.clear_and_free_semaphores(list(self.sems.allocated().values()))
        self.nc.all_engine_barrier()


def _emit_topk_threshold(nc, tk, h_sb, m4):
    """Emit DVE ops computing m4 = 4th-largest-of-each-32-group of h_sb.

    h_sb: [128, 64, 32] f32 tile; m4: [128, 64, 1] f32 tile.
    Exact bitonic partial-merge selection network (validated in numpy)."""
    tt = nc.vector.tensor_tensor

    M = tk.tile([128, 64, 2, 16], F32, tag="tkM")
    # L1: fold halves -> 16 sorted 2-lists (row0=max, row1=min)
    tt(M[:, :, 0, :], h_sb[:, :, 0:16], h_sb[:, :, 16:32], op=MAX)
    tt(M[:, :, 1, :], h_sb[:, :, 0:16], h_sb[:, :, 16:32], op=MIN)

    # L2: Batcher-merge 2-lists (j, j+8) -> 8 sorted 4-lists in T rows S0..S3
    T = tk.tile([128, 64, 4, 8], F32, tag="tkT")
    Q = tk.tile([128, 64, 8], F32, tag="tkQ")
    R = tk.tile([128, 64, 8], F32, tag="tkR")
    tt(T[:, :, 0, :], M[:, :, 0, 0:8], M[:, :, 0, 8:16], op=MAX)
    tt(R[:], M[:, :, 1, 0:8], M[:, :, 1, 8:16], op=MAX)
    tt(Q[:], M[:, :, 0, 0:8], M[:, :, 0, 8:16], op=MIN)
    tt(T[:, :, 3, :], M[:, :, 1, 0:8], M[:, :, 1, 8:16], op=MIN)
    tt(T[:, :, 1, :], Q[:], R[:], op=MAX)
    tt(T[:, :, 2, :], Q[:], R[:], op=MIN)

    # L3/L4: merge sorted-4 list pairs, keep top-4, re-sort (bitonic)
    def merge_level(Tin, w, Uo, Vo, To):
        half = w // 2
        tt(Uo[:], Tin[:, :, :, 0:half], Tin[:, :, ::-1, half:w], op=MAX)
        tt(Vo[:, :, 0:2, :], Uo[:, :, 0:2, :], Uo[:, :, 2:4, :], op=MAX)
        tt(Vo[:, :, 2:4, :], Uo[:, :, 0:2, :], Uo[:, :, 2:4, :], op=MIN)
        tt(To[:, :, 0::2, :], Vo[:, :, 0::2, :], Vo[:, :, 1::2, :], op=MAX)
        tt(To[:, :, 1::2, :], Vo[:, :, 0::2, :], Vo[:, :, 1::2, :], op=MIN)

    # Aggressive buffer aliasing: later (smaller) levels reuse dead regions
    # of earlier buffers so the whole network fits in M, T, Q, R, U + m4.
    U = tk.tile([128, 64, 4, 4], F32, tag="tkU")
    V = T[:, :, :, 0:4]          # T dead after the first merge's U op
    T2 = U[:]                    # U dead once V is built
    merge_level(T[:], 8, U[:], V, T2)

    U2 = Q[:].rearrange("p g (r w) -> p g r w", r=4)   # Q dead after L2
    V2 = R[:].rearrange("p g (r w) -> p g r w", r=4)   # R dead after L2
    Mf = M[:].rearrange("p g r w -> p g (r w)")        # M dead after L2
    T3 = Mf[:, :, 0:8].rearrange("p g (r w) -> p g r w", r=4)
    merge_level(T2, 4, U2, V2, T3)

    # L5: final merge; min of the top-4 multiset = threshold
    U3 = Mf[:, :, 8:12].rearrange("p g (r w) -> p g r w", r=4)
    r2 = Mf[:, :, 12:14].rearrange("p g (r w) -> p g r w", r=2)
    tt(U3, T3[:, :, :, 0:1], T3[:, :, ::-1, 1:2], op=MAX)
    tt(r2, U3[:, :, 0:2, :], U3[:, :, 2:4, :], op=MIN)
    tt(m4[:], r2[:, :, 0, :], r2[:, :, 1, :], op=MIN)


def _build_nc():
    nc = bass.Bass("TRN2", target_bir_lowering=False, debug=False, num_devices=N_CORES)
    # x arrives host-transposed, fp16-scaled, tile-major (layout choice is
    # part of the sharding strategy): xth[t, p, k, j] = f16(64*x[t*128+j, k*128+p])
    xth_d = nc.dram_tensor("xth", [N_TILES, 128, 8, 128], F16, kind="ExternalInput")
    w1h_d = nc.dram_tensor("w1h", [128, 8, PE_DIM], F16, kind="ExternalInput")
    w2h_d = nc.dram_tensor("w2h", [128, 16, OUT_DIM], F16, kind="ExternalInput")
    id_d = nc.dram_tensor("ident", [128, 128], F16, kind="ExternalInput")
    y_d = nc.dram_tensor("y", [TOK_PER_CORE, OUT_DIM], F32, kind="ExternalOutput")
    if PREC2:
        xth8_d = nc.dram_tensor(
            "xth8", [N_TILES, 128, 8, 128], F8, kind="ExternalInput"
        )
        w1l8_d = nc.dram_tensor("w1l8", [128, 8, PE_DIM], F8, kind="ExternalInput")

    A = mybir.AluOpType
    with _TC(nc) as tc:
        with (
            tc.tile_pool(name="weights", bufs=1) as wp,
            tc.tile_pool(name="xp", bufs=3) as xp,
            tc.tile_pool(name="hp", bufs=3) as hp,
            tc.tile_pool(name="h16p", bufs=3) as h16p,
            tc.tile_pool(name="tk", bufs=1) as tk,
            tc.tile_pool(name="zp", bufs=3) as zp,
            tc.tile_pool(name="ztp", bufs=2) as ztp,
            tc.tile_pool(name="op", bufs=2) as op,
            tc.tile_pool(name="psh", bufs=2, space="PSUM") as psh,
            tc.tile_pool(name="pstr", bufs=2, space="PSUM") as pstr,
            tc.tile_pool(name="pso", bufs=2, space="PSUM") as pso,
        ):
            w1h = wp.tile([128, 8, PE_DIM], F16, tag="w1h")
            w2h = wp.tile([128, 16, OUT_DIM], F16, tag="w2h")
            ident = wp.tile([128, 128], F16, tag="ident")
            # weights go on the scalar HWDGE ring so the x-tile DMAs (sync
            # ring) aren't queued behind 7MB of weights at startup
            nc.scalar.dma_start(ident[:], id_d[:])
            # chunked so tile 0's mm1 can chase the DMA
            for kk in range(8):
                nc.scalar.dma_start(w1h[:, kk : kk + 1, :], w1h_d[:, kk : kk + 1, :])
            if PREC2:
                xth8 = [None] * N_TILES
                w1l8 = wp.tile([128, 8, PE_DIM], F8, tag="w1l8")
                for kk in range(4):
                    nc.scalar.dma_start(
                        w1l8[:, 2 * kk : 2 * kk + 2, :],
                        w1l8_d[:, 2 * kk : 2 * kk + 2, :],
                    )
            for c in range(2):
                nc.scalar.dma_start(
                    w2h[:, 8 * c : 8 * c + 8, :], w2h_d[:, 8 * c : 8 * c + 8, :]
                )

            # scratch PSUM target for HAM keep-warm dummy matmuls
            warm_ps = psh.tile([128, 512], F32, tag="warm")

            def keepwarm(n_mm):
                """Dummy ident@ident matmuls to keep the PE clock-gate at
                8/8 through windows where real PE work is sparse."""
                for r in range(n_mm):
                    nc.tensor.matmul(
                        warm_ps[:, 0:128], ident[:], ident[:],
                        start=(r == 0), stop=(r == n_mm - 1),
                    )

            keepwarm(30)  # warm the PE while the first x/w1 DMAs land

            hq = [None] * N_TILES
            h16q = [None] * N_TILES
            zq = [None] * N_TILES

            def stage_a(t):
                """x DMA + fp16 mm1 + PSUM->SBUF h (f32) and h16 copies."""
                xt = xp.tile([128, 8, 128], F16, tag="xt", name=f"xt{t}")
                nc.sync.dma_start(xt[:], xth_d[t])
                if PREC2:
                    x8 = xp.tile([128, 8, 128], F8, tag="xt8", name=f"xt8{t}")
                    nc.sync.dma_start(x8[:], xth8_d[t])
                    xth8[t] = x8
                h_sb = hp.tile([128, 64, 32], F32, tag="h", name=f"h{t}")
                h16 = h16p.tile([128, 64, 32], F16, tag="h16", name=f"h16{t}")
                DR = mybir.MatmulPerfMode.DoubleRow
                for n in range(4):
                    hps = psh.tile([128, 512], F32, tag="hps", name="hps")
                    ncol = slice(n * 512, (n + 1) * 512)
                    for k in range(8):
                        nc.tensor.matmul(
                            hps[:], xt[:, k, :], w1h[:, k, ncol],
                            start=(k == 0), stop=(k == 7) and not PREC2,
                        )
                    if PREC2:
                        for j in range(4):
                            jc = slice(2 * j, 2 * j + 2)
                            nc.tensor.matmul(
                                hps[:], xth8[t][:, jc, :], w1l8[:, jc, ncol],
                                start=False, stop=(j == 3), perf_mode=DR,
                            )
                    gsl = slice(n * 16, (n + 1) * 16)
                    nc.scalar.activation(h_sb[:, gsl, :], hps[:], AF.Copy, scale=HSCALE)
                    nc.scalar.activation(h16[:, gsl, :], hps[:], AF.Copy, scale=HSCALE)
                hq[t] = h_sb
                h16q[t] = h16

            def stage_b(t):
                """DVE: bitonic top-4 threshold, f16 mask, zb = h16*mask."""
                m4 = tk.tile([128, 64, 1], F32, tag="tkm4")
                _emit_topk_threshold(nc, tk, hq[t], m4)
                mask = tk.tile([128, 64, 32], F16, tag="tkmask")
                m4b = m4[:, :, 0].to_broadcast((128, 64, 32))
                nc.vector.tensor_tensor(mask[:], hq[t][:], m4b, op=A.is_ge)
                zb = zp.tile([128, 64, 32], F16, tag="zb", name=f"zb{t}")
                nc.vector.tensor_tensor(zb[:], h16q[t][:], mask[:], op=A.mult)
                zq[t] = zb

            def stage_d(t):
                """z transpose (PE), mm2, output DMA."""
                rows = slice(t * 128, (t + 1) * 128)
                zT = ztp.tile([128, PE_DIM], F16, tag="zT", name=f"zT{t}")
                zbf = zq[t][:].rearrange("p g e -> p (g e)")
                for quad in range(4):
                    pt = pstr.tile([128, 512], F16, tag="tr", name="pt")
                    for q in range(4):
                        k = quad * 4 + q
                        nc.tensor.transpose(
                            pt[:, q * 128 : (q + 1) * 128],
                            zbf[:, k * 128 : (k + 1) * 128],
                            ident[:],
                        )
                    nc.scalar.copy(zT[:, quad * 512 : (quad + 1) * 512], pt[:])

                out_sb = op.tile([128, OUT_DIM], F32, tag="outsb", name=f"o{t}")
                for no in range(2):
                    ops = pso.tile([128, 512], F32, tag="ops", name="ops")
                    ocol = slice(no * 512, (no + 1) * 512)
                    for k in range(16):
                        kc = slice(k * 128, (k + 1) * 128)
                        nc.tensor.matmul(
                            ops[:], zT[:, kc], w2h[:, k, ocol],
                            start=(k == 0), stop=(k == 15),
                        )
                    nc.scalar.copy(out_sb[:, ocol], ops[:])
                nc.sync.dma_start(y_d[rows, :], out_sb[:])

            # 3-stage software pipeline
            for i in range(N_TILES + 2):
                if i < N_TILES:
                    stage_a(i)
                if 0 <= i - 2:
                    stage_d(i - 2)
                    # tail: no more mm1 work; bridge the PE-idle stretches
                    # while the DVE drains so the last tiles' mm2 runs warm
                    if i - 2 == N_TILES - 3:
                        keepwarm(30)
                    elif i - 2 == N_TILES - 2:
                        keepwarm(150)
                if 0 <= i - 1 < N_TILES:
                    stage_b(i - 1)

    return nc


def _prep_inputs(x, w1, w2):
    """Host-side shard + precision-split; returns per-core input maps."""
    x = np.ascontiguousarray(np.asarray(x), dtype=np.float32)
    w1f = np.asarray(w1, dtype=np.float32).reshape(IN_DIM, PE_DIM)
    w2f = np.asarray(w2, dtype=np.float32).reshape(PE_DIM, OUT_DIM)
    xf = x.reshape(-1, IN_DIM)
    assert xf.shape[0] == N_CORES * TOK_PER_CORE

    w1h = np.ascontiguousarray(
        (w1f * 128.0).astype(np.float16).reshape(8, 128, PE_DIM).transpose(1, 0, 2)
    )
    w2h = np.ascontiguousarray(
        w2f.astype(np.float16).reshape(16, 128, OUT_DIM).transpose(1, 0, 2)
    )
    ident = np.eye(128, dtype=np.float16)
    xhT = (xf * 64.0).astype(np.float16).T  # [IN_DIM, N]
    if PREC2:
        import ml_dtypes

        F8NP = ml_dtypes.float8_e4m3fn
        w1l8 = np.ascontiguousarray(
            (
                (w1f - (w1f * 128.0).astype(np.float16).astype(np.float32) / 128.0)
                * 65536.0
            )
            .astype(F8NP)
            .reshape(8, 128, PE_DIM)
            .transpose(1, 0, 2)
        )
        x8T = (xf * 0.125).astype(F8NP).T

    in_maps = []
    for i in range(N_CORES):
        seg = xhT[:, i * TOK_PER_CORE : (i + 1) * TOK_PER_CORE]
        xth = np.ascontiguousarray(
            seg.reshape(8, 128, N_TILES, 128).transpose(2, 1, 0, 3)
        )
        m = {"xth": xth, "w1h": w1h, "w2h": w2h, "ident": ident}
        if PREC2:
            seg8 = x8T[:, i * TOK_PER_CORE : (i + 1) * TOK_PER_CORE]
            m["xth8"] = np.ascontiguousarray(
                seg8.reshape(8, 128, N_TILES, 128).transpose(2, 1, 0, 3)
            )
            m["w1l8"] = w1l8
        in_maps.append(m)
    return in_maps


_NC_CACHE = None


def kernel(x, w1, w2, top_k):
    global _NC_CACHE
    assert int(top_k) == 4
    lead_shape = np.asarray(x).shape[:-1]

    if _NC_CACHE is None:
        _NC_CACHE = _build_nc()
    nc = _NC_CACHE

    in_maps = _prep_inputs(x, w1, w2)
    res = run_bass_kernel_spmd(nc, in_maps, list(range(N_CORES)))
    out = np.concatenate([res.results[i]["y"] for i in range(N_CORES)], axis=0)
    return out.reshape(*lead_shape, OUT_DIM).astype(np.float32)
